# revision 16
# baseline (speedup 1.0000x reference)
"""Trainium2 Bass kernel for nn_ConvLayer_56453050139435.

Reference computation (StyleGAN2-style downsampling conv layer):
  1. depthwise 4x4 binomial blur ([1,3,3,1] outer [1,3,3,1] / 64) with pad 2
  2. 3x3 stride-2 conv, 128 -> 256 channels, weight scaled by 1/sqrt(fan_in)
  3. bias + leaky-relu(0.2) * sqrt(2), clamp +-256 (never binds: |out| < ~4)

Sharding: data-parallel over batch, 2 images per core across 8 cores.

Per-core pipeline (fp16 data path, fp32 PSUM accumulation). The binomial
blur [1,3,3,1] factors as [1,1]*[1,1]*[1,1]; one horizontal [1,1] factor
is folded into the conv weights (kw taps 3 -> 4), so the elementwise blur
is five plain 2-tap adds per strip instead of the naive 4x4 stencil:
  - vertical   [1,1]^3: three adds (s1, s2, V) on DVE (fp16 2x mode)
  - horizontal [1,1]^2 ([1,2,1]): one add on DVE (T), one on GPSIMD (G)
  - conv as 12-tap (3 kh x 4 kw') matmul accumulation in PSUM over G,
    oc split in two 128-halves, rhs = stride-2 access pattern on G
  - epilogue: one ACT Prelu op (scale=sqrt2, bias, alpha=0.2) PSUM->SBUF
  - DMA out fp16 in 16-row batches, host-cast back to fp32

Each block of 16 output rows owns a 36-row strip of the input (4-row halo
recomputed between blocks) so DMA / DVE / GPSIMD / PE / ACT pipeline at
block granularity. Engine budgets per core (cost model): PE 328us (the
bottleneck), DVE 289us, GPSIMD 277us, ACT 73us, DMA 150us.
"""

import numpy as np

import concourse.bass as bass
import concourse.mybir as mybir
from concourse import bacc
from concourse.tile import TileContext
from concourse.bass_utils import run_bass_kernel_spmd

AF = mybir.ActivationFunctionType
OP = mybir.AluOpType
FP16 = mybir.dt.float16
FP32 = mybir.dt.float32

IC, OC, H, W = 128, 256, 256, 256
OH, OW = 128, 128
KS = 3
KW2 = 4          # kw taps after folding one [1,1] blur factor into w
NTAP = 3 * KW2   # 12 accumulation taps per psum tile
N_CORES = 8
B_PER_CORE = 2
SQRT2 = float(np.sqrt(2.0))
WSCALE = 1.0 / float(np.sqrt(KS * KS * IC))
LRELU_SLOPE = 0.2

SROWS = 36       # x rows per strip (32 blurred rows + 4-row halo)
VROWS = 33       # blurred rows per strip
NPAIR = 8        # strips per image (16 output rows each)
VP = 260         # V tile width: cols 0..1 / 258..259 are zero padding


def _build_nc():
    nc = bacc.Bacc(None, target_bir_lowering=False)
    x_d = nc.dram_tensor("x", [B_PER_CORE, IC, H, W], FP16, kind="ExternalInput")
    w_d = nc.dram_tensor("w", [IC, 2 * NTAP * 128], FP16, kind="ExternalInput")
    b_d = nc.dram_tensor("b", [128, 2], FP32, kind="ExternalInput")
    y_d = nc.dram_tensor("y", [B_PER_CORE, OC, OH, OW], FP16, kind="ExternalOutput")

    with TileContext(nc) as tc:
        with (
            tc.tile_pool(name="const", bufs=1) as cpool,
            tc.tile_pool(name="xin", bufs=2) as xpool,
            tc.tile_pool(name="scr", bufs=1) as scrpool,
            tc.tile_pool(name="vst", bufs=1) as vpool,
            tc.tile_pool(name="tst", bufs=2) as tpool,
            tc.tile_pool(name="gst", bufs=2) as gpool,
            tc.tile_pool(name="out", bufs=4) as opool,
            tc.tile_pool(name="psum", bufs=8, space="PSUM") as pspool,
        ):
            wt = cpool.tile([128, 2 * NTAP * 128], FP16)
            bt = cpool.tile([128, 2], FP32)
            al = cpool.tile([128, 1], FP32)
            consts_loaded = False
            nc.vector.memset(al[:], LRELU_SLOPE)

            # DVE-only scratch: single-buffered (engine order serializes)
            s1 = scrpool.tile([128, SROWS - 1, W], FP16)
            s2 = scrpool.tile([128, SROWS - 2, W], FP16)

            for img in range(B_PER_CORE):
                for P in range(NPAIR):
                    lo = 32 * P - 2       # x row of strip row 0
                    xr0 = max(lo, 0)
                    xr1 = min(lo + SROWS, H)
                    ta, tb = xr0 - lo, xr1 - lo  # valid strip row range

                    first = img == 0 and P <= 1
                    last = img == B_PER_CORE - 1 and P == NPAIR - 1
                    xt = xpool.tile([128, SROWS, W], FP16)
                    if ta > 0:
                        nc.gpsimd.memset(xt[:, 0:ta, :], 0.0)
                    if tb < SROWS:
                        nc.gpsimd.memset(xt[:, tb:SROWS, :], 0.0)
                    if first:
                        # split the early strips' loads so the first blur
                        # segment starts as soon as its rows land
                        nc.sync.dma_start(
                            xt[:, ta:13, :], x_d[img, :, xr0 : lo + 13, :]
                        )
                        nc.sync.dma_start(
                            xt[:, 13:tb, :], x_d[img, :, lo + 13 : xr1, :]
                        )
                    else:
                        nc.sync.dma_start(
                            xt[:, ta:tb, :], x_d[img, :, xr0:xr1, :]
                        )
                    if not consts_loaded:
                        # weights load behind the first x strip: they are not
                        # needed until the first matmul ~16us in
                        consts_loaded = True
                        nc.sync.dma_start(wt[:], w_d[:])
                        nc.sync.dma_start(bt[:], b_d[:])

                    # blur passes, each split into two row-halves over the
                    # same strip tiles (disjoint rows, no recompute) so the
                    # first psum group's G rows are ready ~17us earlier
                    vt = vpool.tile([128, VROWS, VP], FP16)
                    tt = tpool.tile([128, VROWS, VP - 1], FP16)
                    gt = gpool.tile([128, VROWS, VP - 2], FP16)
                    nc.gpsimd.memset(vt[:, :, 0:2], 0.0)
                    nc.gpsimd.memset(vt[:, :, VP - 2 : VP], 0.0)
                    # half A covers G rows 0..16 (psum groups g=0,1), half B
                    # rows 17..32 (g=2,3 read G row 16 from half A); each
                    # pass's halves write disjoint rows, cross-half reads hit
                    # the row the other half wrote into the shared tile. The
                    # very first strip runs in quarters to shrink the
                    # pipeline-fill time before the first matmul.
                    if first or last:
                        segs = (
                            ((0, 11), (0, 10), (0, 9)),
                            ((11, 19), (10, 18), (9, 17)),
                            ((19, 27), (18, 26), (17, 25)),
                            ((27, 35), (26, 34), (25, 33)),
                        )
                        gsets = ((0,), (1,), (2,), (3,))
                    else:
                        segs = (
                            ((0, 19), (0, 18), (0, 17)),
                            ((19, SROWS - 1), (18, SROWS - 2), (17, VROWS)),
                        )
                        gsets = ((0, 1), (2, 3))
                    for (a1, b1), (a2, b2), (av, bv) in segs:
                        # vertical [1,3,3,1] = three [1,1] passes (DVE 2x)
                        nc.vector.tensor_tensor(
                            out=s1[:, a1:b1, :], in0=xt[:, a1:b1, :],
                            in1=xt[:, a1 + 1 : b1 + 1, :], op=OP.add,
                        )
                        nc.vector.tensor_tensor(
                            out=s2[:, a2:b2, :], in0=s1[:, a2:b2, :],
                            in1=s1[:, a2 + 1 : b2 + 1, :], op=OP.add,
                        )
                        nc.vector.tensor_tensor(
                            out=vt[:, av:bv, 2 : 2 + W], in0=s2[:, av:bv, :],
                            in1=s2[:, av + 1 : bv + 1, :], op=OP.add,
                        )
                        # horizontal [1,2,1] = two [1,1] passes (DVE, GPSIMD);
                        # the third [1,1] factor lives in the conv weights
                        nc.vector.tensor_tensor(
                            out=tt[:, av:bv, :], in0=vt[:, av:bv, 0 : VP - 1],
                            in1=vt[:, av:bv, 1:VP], op=OP.add,
                        )
                        # the final [1,1] pass runs on GPSIMD in steady state;
                        # during pipeline fill it runs on DVE, which is idle
                        # there and ~3.6x faster per element
                        h2_eng = nc.vector if first else nc.gpsimd
                        h2_eng.tensor_tensor(
                            out=gt[:, av:bv, :], in0=tt[:, av:bv, 0 : VP - 2],
                            in1=tt[:, av:bv, 1 : VP - 1], op=OP.add,
                        )

                    # conv: 12-tap PSUM accumulation per 4-row out group;
                    # half-A groups (g=0,1) issue before half-B groups
                    p0 = 16 * P
                    ots = [
                        opool.tile([128, 16, OW], FP16, name=f"ot{h}")
                        for h in range(2)
                    ]
                    for gset in gsets:
                        r0, r1 = 4 * gset[0], 4 * gset[-1] + 4
                        for oc_h in range(2):
                            for g in gset:
                                ps = pspool.tile([128, 4, OW], FP32)
                                for t in range(NTAP):
                                    kh, kw = t // KW2, t % KW2
                                    idx = t * 2 + oc_h
                                    nc.tensor.matmul(
                                        ps[:],
                                        wt[:, idx * 128 : (idx + 1) * 128],
                                        gt[:, 8 * g + kh : 8 * g + kh + 7 : 2,
                                           kw : kw + 255 : 2],
                                        start=(t == 0),
                                        stop=(t == NTAP - 1),
                                    )
                                nc.scalar.activation(
                                    ots[oc_h][:, 4 * g : 4 * g + 4, :], ps[:],
                                    AF.Prelu,
                                    bias=bt[:, oc_h : oc_h + 1],
                                    scale=SQRT2,
                                    alpha=al[:, 0:1],
                                )
                            # ship each finished row span immediately so the
                            # store never sits behind the next strip's compute
                            nc.sync.dma_start(
                                y_d[img, 128 * oc_h : 128 * (oc_h + 1),
                                    p0 + r0 : p0 + r1, :],
                                ots[oc_h][:, r0:r1, :],
                            )
    nc.finalize()
    return nc


_NC = None


def _get_nc():
    global _NC
    if _NC is None:
        _NC = _build_nc()
    return _NC


def kernel(x, weight, bias):
    x = np.asarray(x, dtype=np.float32)
    weight = np.asarray(weight, dtype=np.float32)
    bias = np.asarray(bias, dtype=np.float32)

    # host-side prep: fold wscale and the blur's 1/64 norm into the weights,
    # plus the [1,1] horizontal blur factor (kw taps 3 -> 4); sqrt(2) gain
    # into the bias; lay out lhsT tiles per (tap, half)
    w_eff = weight * (WSCALE / 64.0)                      # [256,128,3,3]
    w2 = np.zeros((OC, IC, KS, KW2), dtype=np.float32)
    w2[:, :, :, 0:3] += w_eff
    w2[:, :, :, 1:4] += w_eff
    w2 = w2.astype(np.float16)
    w_sb = np.empty((IC, 2 * NTAP * 128), dtype=np.float16)
    for t in range(NTAP):
        kh, kw = t // KW2, t % KW2
        for oc_h in range(2):
            idx = t * 2 + oc_h
            w_sb[:, idx * 128 : (idx + 1) * 128] = (
                w2[oc_h * 128 : (oc_h + 1) * 128, :, kh, kw].T
            )
    b_sb = (SQRT2 * bias).astype(np.float32).reshape(2, 128).T.copy()  # [128,2]

    x16 = x.astype(np.float16)
    nc = _get_nc()
    in_maps = [
        {
            "x": x16[c * B_PER_CORE : (c + 1) * B_PER_CORE],
            "w": w_sb,
            "b": b_sb,
        }
        for c in range(N_CORES)
    ]
    res = run_bass_kernel_spmd(nc, in_maps, core_ids=list(range(N_CORES)))
    y16 = np.concatenate([res.results[c]["y"] for c in range(N_CORES)], axis=0)
    return y16.astype(np.float32)


# revision 17
# speedup vs baseline: 1.0191x; 1.0191x over previous
"""Trainium2 Bass kernel for nn_ConvLayer_56453050139435.

Reference computation (StyleGAN2-style downsampling conv layer):
  1. depthwise 4x4 binomial blur ([1,3,3,1] outer [1,3,3,1] / 64) with pad 2
  2. 3x3 stride-2 conv, 128 -> 256 channels, weight scaled by 1/sqrt(fan_in)
  3. bias + leaky-relu(0.2) * sqrt(2), clamp +-256 (never binds: |out| < ~4)

Sharding: data-parallel over batch, 2 images per core across 8 cores.

Per-core pipeline (fp16 data path, fp32 PSUM accumulation). The binomial
blur [1,3,3,1] factors as [1,1]*[1,1]*[1,1]; one horizontal [1,1] factor
is folded into the conv weights (kw taps 3 -> 4), so the elementwise blur
is five plain 2-tap adds per strip instead of the naive 4x4 stencil:
  - vertical   [1,1]^3: three adds (s1, s2, V) on DVE (fp16 2x mode)
  - horizontal [1,1]^2 ([1,2,1]): one add on DVE (T), one on GPSIMD (G)
  - conv as 12-tap (3 kh x 4 kw') matmul accumulation in PSUM over G,
    oc split in two 128-halves, rhs = stride-2 access pattern on G
  - epilogue: one ACT Prelu op (scale=sqrt2, bias, alpha=0.2) PSUM->SBUF
  - DMA out fp16 in 16-row batches, host-cast back to fp32

Each block of 16 output rows owns a 36-row strip of the input (4-row halo
recomputed between blocks) so DMA / DVE / GPSIMD / PE / ACT pipeline at
block granularity. Engine budgets per core (cost model): PE 328us (the
bottleneck), DVE 289us, GPSIMD 277us, ACT 73us, DMA 150us.
"""

import numpy as np

import concourse.bass as bass
import concourse.mybir as mybir
from concourse import bacc
from concourse.tile import TileContext
from concourse.bass_utils import run_bass_kernel_spmd

AF = mybir.ActivationFunctionType
OP = mybir.AluOpType
FP16 = mybir.dt.float16
FP32 = mybir.dt.float32

IC, OC, H, W = 128, 256, 256, 256
OH, OW = 128, 128
KS = 3
KW2 = 4          # kw taps after folding one [1,1] blur factor into w
NTAP = 3 * KW2   # 12 accumulation taps per psum tile
N_CORES = 8
B_PER_CORE = 2
SQRT2 = float(np.sqrt(2.0))
WSCALE = 1.0 / float(np.sqrt(KS * KS * IC))
LRELU_SLOPE = 0.2

SROWS = 36       # x rows per strip (32 blurred rows + 4-row halo)
VROWS = 33       # blurred rows per strip
NPAIR = 8        # strips per image (16 output rows each)
VP = 260         # V tile width: cols 0..1 / 258..259 are zero padding


def _build_nc():
    nc = bacc.Bacc(None, target_bir_lowering=False)
    x_d = nc.dram_tensor("x", [B_PER_CORE, IC, H, W], FP16, kind="ExternalInput")
    w_d = nc.dram_tensor("w", [IC, 2 * NTAP * 128], FP16, kind="ExternalInput")
    b_d = nc.dram_tensor("b", [128, 2], FP32, kind="ExternalInput")
    y_d = nc.dram_tensor("y", [B_PER_CORE, OC, OH, OW], FP16, kind="ExternalOutput")

    with TileContext(nc) as tc:
        with (
            tc.tile_pool(name="const", bufs=1) as cpool,
            tc.tile_pool(name="xin", bufs=2) as xpool,
            tc.tile_pool(name="scr", bufs=1) as scrpool,
            tc.tile_pool(name="vst", bufs=1) as vpool,
            tc.tile_pool(name="tst", bufs=2) as tpool,
            tc.tile_pool(name="gst", bufs=2) as gpool,
            tc.tile_pool(name="out", bufs=4) as opool,
            tc.tile_pool(name="psum", bufs=8, space="PSUM") as pspool,
        ):
            wt = cpool.tile([128, 2 * NTAP * 128], FP16)
            bt = cpool.tile([128, 2], FP32)
            al = cpool.tile([128, 1], FP32)
            consts_loaded = False
            nc.vector.memset(al[:], LRELU_SLOPE)

            # DVE-only scratch: single-buffered (engine order serializes)
            s1 = scrpool.tile([128, SROWS - 1, W], FP16)
            s2 = scrpool.tile([128, SROWS - 2, W], FP16)

            for img in range(B_PER_CORE):
                for P in range(NPAIR):
                    lo = 32 * P - 2       # x row of strip row 0
                    xr0 = max(lo, 0)
                    xr1 = min(lo + SROWS, H)
                    ta, tb = xr0 - lo, xr1 - lo  # valid strip row range

                    first = img == 0 and P <= 1
                    last = img == B_PER_CORE - 1 and P == NPAIR - 1
                    xt = xpool.tile([128, SROWS, W], FP16)
                    if ta > 0:
                        nc.gpsimd.memset(xt[:, 0:ta, :], 0.0)
                    if tb < SROWS:
                        nc.gpsimd.memset(xt[:, tb:SROWS, :], 0.0)
                    if first:
                        # split the early strips' loads so the first blur
                        # segment starts as soon as its rows land
                        nc.sync.dma_start(
                            xt[:, ta:13, :], x_d[img, :, xr0 : lo + 13, :]
                        )
                        nc.sync.dma_start(
                            xt[:, 13:tb, :], x_d[img, :, lo + 13 : xr1, :]
                        )
                    else:
                        nc.sync.dma_start(
                            xt[:, ta:tb, :], x_d[img, :, xr0:xr1, :]
                        )
                    if not consts_loaded:
                        # weights load behind the first x strip: they are not
                        # needed until the first matmul ~16us in
                        consts_loaded = True
                        nc.sync.dma_start(wt[:], w_d[:])
                        nc.sync.dma_start(bt[:], b_d[:])

                    # blur passes, each split into two row-halves over the
                    # same strip tiles (disjoint rows, no recompute) so the
                    # first psum group's G rows are ready ~17us earlier
                    vt = vpool.tile([128, VROWS, VP], FP16)
                    tt = tpool.tile([128, VROWS, VP - 1], FP16)
                    gt = gpool.tile([128, VROWS, VP - 2], FP16)
                    nc.gpsimd.memset(vt[:, :, 0:2], 0.0)
                    nc.gpsimd.memset(vt[:, :, VP - 2 : VP], 0.0)
                    # half A covers G rows 0..16 (psum groups g=0,1), half B
                    # rows 17..32 (g=2,3 read G row 16 from half A); each
                    # pass's halves write disjoint rows, cross-half reads hit
                    # the row the other half wrote into the shared tile. The
                    # very first strip runs in quarters to shrink the
                    # pipeline-fill time before the first matmul.
                    if first or last:
                        segs = (
                            ((0, 11), (0, 10), (0, 9)),
                            ((11, 19), (10, 18), (9, 17)),
                            ((19, 27), (18, 26), (17, 25)),
                            ((27, 35), (26, 34), (25, 33)),
                        )
                        gsets = ((0,), (1,), (2,), (3,))
                    else:
                        segs = (
                            ((0, 19), (0, 18), (0, 17)),
                            ((19, SROWS - 1), (18, SROWS - 2), (17, VROWS)),
                        )
                        gsets = ((0, 1), (2, 3))
                    for (a1, b1), (a2, b2), (av, bv) in segs:
                        # vertical [1,3,3,1] = three [1,1] passes (DVE 2x)
                        nc.vector.tensor_tensor(
                            out=s1[:, a1:b1, :], in0=xt[:, a1:b1, :],
                            in1=xt[:, a1 + 1 : b1 + 1, :], op=OP.add,
                        )
                        nc.vector.tensor_tensor(
                            out=s2[:, a2:b2, :], in0=s1[:, a2:b2, :],
                            in1=s1[:, a2 + 1 : b2 + 1, :], op=OP.add,
                        )
                        nc.vector.tensor_tensor(
                            out=vt[:, av:bv, 2 : 2 + W], in0=s2[:, av:bv, :],
                            in1=s2[:, av + 1 : bv + 1, :], op=OP.add,
                        )
                        # horizontal [1,2,1] = two [1,1] passes (DVE, GPSIMD);
                        # the third [1,1] factor lives in the conv weights
                        nc.vector.tensor_tensor(
                            out=tt[:, av:bv, :], in0=vt[:, av:bv, 0 : VP - 1],
                            in1=vt[:, av:bv, 1:VP], op=OP.add,
                        )
                        # the final [1,1] pass runs on GPSIMD in steady state;
                        # for the very first segment it runs on DVE, which is
                        # idle there and ~3.6x faster per element
                        h2_eng = (
                            nc.vector
                            if (img == 0 and P == 0 and av == 0)
                            else nc.gpsimd
                        )
                        h2_eng.tensor_tensor(
                            out=gt[:, av:bv, :], in0=tt[:, av:bv, 0 : VP - 2],
                            in1=tt[:, av:bv, 1 : VP - 1], op=OP.add,
                        )

                    # conv: 12-tap PSUM accumulation per 4-row out group;
                    # half-A groups (g=0,1) issue before half-B groups
                    p0 = 16 * P
                    ots = [
                        opool.tile([128, 16, OW], FP16, name=f"ot{h}")
                        for h in range(2)
                    ]
                    for gset in gsets:
                        r0, r1 = 4 * gset[0], 4 * gset[-1] + 4
                        for oc_h in range(2):
                            for g in gset:
                                ps = pspool.tile([128, 4, OW], FP32)
                                for t in range(NTAP):
                                    kh, kw = t // KW2, t % KW2
                                    idx = t * 2 + oc_h
                                    nc.tensor.matmul(
                                        ps[:],
                                        wt[:, idx * 128 : (idx + 1) * 128],
                                        gt[:, 8 * g + kh : 8 * g + kh + 7 : 2,
                                           kw : kw + 255 : 2],
                                        start=(t == 0),
                                        stop=(t == NTAP - 1),
                                    )
                                nc.scalar.activation(
                                    ots[oc_h][:, 4 * g : 4 * g + 4, :], ps[:],
                                    AF.Prelu,
                                    bias=bt[:, oc_h : oc_h + 1],
                                    scale=SQRT2,
                                    alpha=al[:, 0:1],
                                )
                            # ship each finished row span immediately so the
                            # store never sits behind the next strip's compute
                            nc.sync.dma_start(
                                y_d[img, 128 * oc_h : 128 * (oc_h + 1),
                                    p0 + r0 : p0 + r1, :],
                                ots[oc_h][:, r0:r1, :],
                            )
    nc.finalize()
    return nc


_NC = None


def _get_nc():
    global _NC
    if _NC is None:
        _NC = _build_nc()
    return _NC


def kernel(x, weight, bias):
    x = np.asarray(x, dtype=np.float32)
    weight = np.asarray(weight, dtype=np.float32)
    bias = np.asarray(bias, dtype=np.float32)

    # host-side prep: fold wscale and the blur's 1/64 norm into the weights,
    # plus the [1,1] horizontal blur factor (kw taps 3 -> 4); sqrt(2) gain
    # into the bias; lay out lhsT tiles per (tap, half)
    w_eff = weight * (WSCALE / 64.0)                      # [256,128,3,3]
    w2 = np.zeros((OC, IC, KS, KW2), dtype=np.float32)
    w2[:, :, :, 0:3] += w_eff
    w2[:, :, :, 1:4] += w_eff
    w2 = w2.astype(np.float16)
    w_sb = np.empty((IC, 2 * NTAP * 128), dtype=np.float16)
    for t in range(NTAP):
        kh, kw = t // KW2, t % KW2
        for oc_h in range(2):
            idx = t * 2 + oc_h
            w_sb[:, idx * 128 : (idx + 1) * 128] = (
                w2[oc_h * 128 : (oc_h + 1) * 128, :, kh, kw].T
            )
    b_sb = (SQRT2 * bias).astype(np.float32).reshape(2, 128).T.copy()  # [128,2]

    x16 = x.astype(np.float16)
    nc = _get_nc()
    in_maps = [
        {
            "x": x16[c * B_PER_CORE : (c + 1) * B_PER_CORE],
            "w": w_sb,
            "b": b_sb,
        }
        for c in range(N_CORES)
    ]
    res = run_bass_kernel_spmd(nc, in_maps, core_ids=list(range(N_CORES)))
    y16 = np.concatenate([res.results[c]["y"] for c in range(N_CORES)], axis=0)
    return y16.astype(np.float32)


# revision 18
# speedup vs baseline: 1.0455x; 1.0260x over previous
"""Trainium2 Bass kernel for nn_ConvLayer_56453050139435.

Reference computation (StyleGAN2-style downsampling conv layer):
  1. depthwise 4x4 binomial blur ([1,3,3,1] outer [1,3,3,1] / 64) with pad 2
  2. 3x3 stride-2 conv, 128 -> 256 channels, weight scaled by 1/sqrt(fan_in)
  3. bias + leaky-relu(0.2) * sqrt(2), clamp +-256 (never binds: |out| < ~4)

Sharding: data-parallel over batch, 2 images per core across 8 cores.

Per-core pipeline (fp16 data path, fp32 PSUM accumulation). The binomial
blur [1,3,3,1] factors as [1,1]*[1,1]*[1,1]; one horizontal [1,1] factor
is folded into the conv weights (kw taps 3 -> 4), so the elementwise blur
is five plain 2-tap adds per strip instead of the naive 4x4 stencil:
  - vertical   [1,1]^3: three adds (s1, s2, V) on DVE (fp16 2x mode)
  - horizontal [1,1]^2 ([1,2,1]): one add on DVE (T), one on GPSIMD (G)
  - conv as 12-tap (3 kh x 4 kw') matmul accumulation in PSUM over G,
    oc split in two 128-halves, rhs = stride-2 access pattern on G
  - epilogue: one ACT Prelu op (scale=sqrt2, bias, alpha=0.2) PSUM->SBUF
  - DMA out fp16 in 16-row batches, host-cast back to fp32

Each block of 16 output rows owns a 36-row strip of the input (4-row halo
recomputed between blocks) so DMA / DVE / GPSIMD / PE / ACT pipeline at
block granularity. Engine budgets per core (cost model): PE 328us (the
bottleneck), DVE 289us, GPSIMD 277us, ACT 73us, DMA 150us.
"""

import numpy as np

import concourse.bass as bass
import concourse.mybir as mybir
from concourse import bacc
from concourse.tile import TileContext
from concourse.bass_utils import run_bass_kernel_spmd

AF = mybir.ActivationFunctionType
OP = mybir.AluOpType
FP16 = mybir.dt.float16
FP32 = mybir.dt.float32

IC, OC, H, W = 128, 256, 256, 256
OH, OW = 128, 128
KS = 3
KW2 = 4          # kw taps after folding one [1,1] blur factor into w
NTAP = 3 * KW2   # 12 accumulation taps per psum tile
N_CORES = 8
B_PER_CORE = 2
SQRT2 = float(np.sqrt(2.0))
WSCALE = 1.0 / float(np.sqrt(KS * KS * IC))
LRELU_SLOPE = 0.2

SROWS = 36       # x rows per strip (32 blurred rows + 4-row halo)
VROWS = 33       # blurred rows per strip
NPAIR = 8        # strips per image (16 output rows each)
VP = 260         # V tile width: cols 0..1 / 258..259 are zero padding


def _build_nc():
    nc = bacc.Bacc(None, target_bir_lowering=False)
    x_d = nc.dram_tensor("x", [B_PER_CORE, IC, H, W], FP16, kind="ExternalInput")
    w_d = nc.dram_tensor("w", [IC, 2 * NTAP * 128], FP16, kind="ExternalInput")
    b_d = nc.dram_tensor("b", [128, 2], FP32, kind="ExternalInput")
    y_d = nc.dram_tensor("y", [B_PER_CORE, OC, OH, OW], FP16, kind="ExternalOutput")

    with TileContext(nc) as tc:
        with (
            tc.tile_pool(name="const", bufs=1) as cpool,
            tc.tile_pool(name="xin", bufs=2) as xpool,
            tc.tile_pool(name="scr", bufs=1) as scrpool,
            tc.tile_pool(name="vst", bufs=1) as vpool,
            tc.tile_pool(name="tst", bufs=2) as tpool,
            tc.tile_pool(name="gst", bufs=2) as gpool,
            tc.tile_pool(name="out", bufs=4) as opool,
            tc.tile_pool(name="psum", bufs=8, space="PSUM") as pspool,
        ):
            wt = cpool.tile([128, 2 * NTAP * 128], FP16)
            bt = cpool.tile([128, 2], FP32)
            al = cpool.tile([128, 1], FP32)
            consts_loaded = False
            nc.vector.memset(al[:], LRELU_SLOPE)

            # DVE-only scratch: single-buffered (engine order serializes)
            s1 = scrpool.tile([128, SROWS - 1, W], FP16)
            s2 = scrpool.tile([128, SROWS - 2, W], FP16)

            for img in range(B_PER_CORE):
                for P in range(NPAIR):
                    lo = 32 * P - 2       # x row of strip row 0
                    xr0 = max(lo, 0)
                    xr1 = min(lo + SROWS, H)
                    ta, tb = xr0 - lo, xr1 - lo  # valid strip row range

                    first = img == 0 and P <= 1
                    last = img == B_PER_CORE - 1 and P == NPAIR - 1
                    xt = xpool.tile([128, SROWS, W], FP16)
                    if ta > 0:
                        nc.gpsimd.memset(xt[:, 0:ta, :], 0.0)
                    if tb < SROWS:
                        nc.gpsimd.memset(xt[:, tb:SROWS, :], 0.0)
                    if first:
                        # split the early strips' loads so the first blur
                        # segment starts as soon as its rows land
                        nc.sync.dma_start(
                            xt[:, ta:13, :], x_d[img, :, xr0 : lo + 13, :]
                        )
                        nc.sync.dma_start(
                            xt[:, 13:tb, :], x_d[img, :, lo + 13 : xr1, :]
                        )
                    else:
                        nc.sync.dma_start(
                            xt[:, ta:tb, :], x_d[img, :, xr0:xr1, :]
                        )
                    if not consts_loaded:
                        # weights load behind the first x strip: they are not
                        # needed until the first matmul ~16us in
                        consts_loaded = True
                        nc.sync.dma_start(wt[:], w_d[:])
                        nc.sync.dma_start(bt[:], b_d[:])

                    # blur passes, each split into two row-halves over the
                    # same strip tiles (disjoint rows, no recompute) so the
                    # first psum group's G rows are ready ~17us earlier
                    vt = vpool.tile([128, VROWS, VP], FP16)
                    tt = tpool.tile([128, VROWS, VP - 1], FP16)
                    gt = gpool.tile([128, VROWS, VP - 2], FP16)
                    nc.gpsimd.memset(vt[:, :, 0:2], 0.0)
                    nc.gpsimd.memset(vt[:, :, VP - 2 : VP], 0.0)
                    # half A covers G rows 0..16 (psum groups g=0,1), half B
                    # rows 17..32 (g=2,3 read G row 16 from half A); each
                    # pass's halves write disjoint rows, cross-half reads hit
                    # the row the other half wrote into the shared tile. The
                    # very first strip runs in quarters to shrink the
                    # pipeline-fill time before the first matmul.
                    if first or last:
                        segs = (
                            ((0, 11), (0, 10), (0, 9)),
                            ((11, 19), (10, 18), (9, 17)),
                            ((19, 27), (18, 26), (17, 25)),
                            ((27, 35), (26, 34), (25, 33)),
                        )
                        gsets = ((0,), (1,), (2,), (3,))
                    else:
                        segs = (
                            ((0, 19), (0, 18), (0, 17)),
                            ((19, SROWS - 1), (18, SROWS - 2), (17, VROWS)),
                        )
                        gsets = ((0, 1), (2, 3))
                    for (a1, b1), (a2, b2), (av, bv) in segs:
                        # vertical [1,3,3,1] = three [1,1] passes (DVE 2x)
                        nc.vector.tensor_tensor(
                            out=s1[:, a1:b1, :], in0=xt[:, a1:b1, :],
                            in1=xt[:, a1 + 1 : b1 + 1, :], op=OP.add,
                        )
                        nc.vector.tensor_tensor(
                            out=s2[:, a2:b2, :], in0=s1[:, a2:b2, :],
                            in1=s1[:, a2 + 1 : b2 + 1, :], op=OP.add,
                        )
                        nc.vector.tensor_tensor(
                            out=vt[:, av:bv, 2 : 2 + W], in0=s2[:, av:bv, :],
                            in1=s2[:, av + 1 : bv + 1, :], op=OP.add,
                        )
                        # horizontal [1,2,1] = two [1,1] passes (DVE, GPSIMD);
                        # the third [1,1] factor lives in the conv weights
                        nc.vector.tensor_tensor(
                            out=tt[:, av:bv, :], in0=vt[:, av:bv, 0 : VP - 1],
                            in1=vt[:, av:bv, 1:VP], op=OP.add,
                        )
                        nc.gpsimd.tensor_tensor(
                            out=gt[:, av:bv, :], in0=tt[:, av:bv, 0 : VP - 2],
                            in1=tt[:, av:bv, 1 : VP - 1], op=OP.add,
                        )

                    # conv: 12-tap PSUM accumulation per 4-row out group;
                    # half-A groups (g=0,1) issue before half-B groups
                    p0 = 16 * P
                    ots = [
                        opool.tile([128, 16, OW], FP16, name=f"ot{h}")
                        for h in range(2)
                    ]
                    for gset in gsets:
                        r0, r1 = 4 * gset[0], 4 * gset[-1] + 4
                        for oc_h in range(2):
                            for g in gset:
                                ps = pspool.tile([128, 4, OW], FP32)
                                for t in range(NTAP):
                                    kh, kw = t // KW2, t % KW2
                                    idx = t * 2 + oc_h
                                    nc.tensor.matmul(
                                        ps[:],
                                        wt[:, idx * 128 : (idx + 1) * 128],
                                        gt[:, 8 * g + kh : 8 * g + kh + 7 : 2,
                                           kw : kw + 255 : 2],
                                        start=(t == 0),
                                        stop=(t == NTAP - 1),
                                    )
                                nc.scalar.activation(
                                    ots[oc_h][:, 4 * g : 4 * g + 4, :], ps[:],
                                    AF.Prelu,
                                    bias=bt[:, oc_h : oc_h + 1],
                                    scale=SQRT2,
                                    alpha=al[:, 0:1],
                                )
                            # ship each finished row span immediately so the
                            # store never sits behind the next strip's compute
                            nc.sync.dma_start(
                                y_d[img, 128 * oc_h : 128 * (oc_h + 1),
                                    p0 + r0 : p0 + r1, :],
                                ots[oc_h][:, r0:r1, :],
                            )
    nc.finalize()
    return nc


_NC = None


def _get_nc():
    global _NC
    if _NC is None:
        _NC = _build_nc()
    return _NC


def kernel(x, weight, bias):
    x = np.asarray(x, dtype=np.float32)
    weight = np.asarray(weight, dtype=np.float32)
    bias = np.asarray(bias, dtype=np.float32)

    # host-side prep: fold wscale and the blur's 1/64 norm into the weights,
    # plus the [1,1] horizontal blur factor (kw taps 3 -> 4); sqrt(2) gain
    # into the bias; lay out lhsT tiles per (tap, half)
    w_eff = weight * (WSCALE / 64.0)                      # [256,128,3,3]
    w2 = np.zeros((OC, IC, KS, KW2), dtype=np.float32)
    w2[:, :, :, 0:3] += w_eff
    w2[:, :, :, 1:4] += w_eff
    w2 = w2.astype(np.float16)
    w_sb = np.empty((IC, 2 * NTAP * 128), dtype=np.float16)
    for t in range(NTAP):
        kh, kw = t // KW2, t % KW2
        for oc_h in range(2):
            idx = t * 2 + oc_h
            w_sb[:, idx * 128 : (idx + 1) * 128] = (
                w2[oc_h * 128 : (oc_h + 1) * 128, :, kh, kw].T
            )
    b_sb = (SQRT2 * bias).astype(np.float32).reshape(2, 128).T.copy()  # [128,2]

    x16 = x.astype(np.float16)
    nc = _get_nc()
    in_maps = [
        {
            "x": x16[c * B_PER_CORE : (c + 1) * B_PER_CORE],
            "w": w_sb,
            "b": b_sb,
        }
        for c in range(N_CORES)
    ]
    res = run_bass_kernel_spmd(nc, in_maps, core_ids=list(range(N_CORES)))
    y16 = np.concatenate([res.results[c]["y"] for c in range(N_CORES)], axis=0)
    return y16.astype(np.float32)
